# revision 1
# baseline (speedup 1.0000x reference)
# Trainium2 Bass kernel for nn_LocalCrossAttentionModule.
#
# Math: softmax over a size-1 axis is identically 1, so q/k (and x_query,
# Wq, bq, Wk, bk) never affect the output. The module reduces to, per
# 5x5 patch p (576 of them = 4 batch x 12x12 grid, stride 36):
#   kvf_p  = flatten(x_kv patch)                  (3200,)
#   v_p    = Wv @ kvf_p + bv                      (1600,) viewed as (64, 5, 5)
#   z_p    = conv_w @ v_p[:, s] + conv_b          (128,) per pixel s in 5x5
# z_p is scattered into an otherwise-constant (conv_b) output map.
#
# Sharding: the 25 patch pixels s are split across 8 cores (4 slots each,
# 7 junk/dup slots). Every core sees all 576 patches as the matmul moving
# dim (2 chunks of 288 >= 256 keeps float32r matmuls at full rate).
# Host does layout only: patch gather, weight permutation/transpose,
# final scatter into the conv_b-filled canvas.

import numpy as np

B = 4
CKV = 128
HW_ = 432
E = 2
PP = 5          # patch side
STRIDE = 36
PI = 12         # patch grid side
NP = B * PI * PI   # 576 patches
KF = CKV * PP * PP  # 3200 kv features per patch
KT = KF // 128      # 25 contraction tiles
OUT = 64
O2 = 128
SLOTS = 4
M = SLOTS * OUT    # 256 v-features per core
NCHUNK = 288       # patch chunk (2 x 288 = 576)
NCORES = 8

DTYPE = "f16"      # "f32r" (most accurate) | "f16" (half DMA bytes, ~5e-4) | "bf16"

# pixel-slot assignment: cores 0-6 own 3 pixels (4th slot duplicates the
# first), core 7 owns 4.
S_LISTS = [[3 * c, 3 * c + 1, 3 * c + 2, 3 * c] for c in range(7)]
S_LISTS.append([21, 22, 23, 24])
VALID = [3] * 7 + [4]

_PROGRAM = {}


def _build_program(dtype=DTYPE):
    import concourse.mybir as mybir
    from concourse import bacc
    from concourse.tile import TileContext

    f32 = mybir.dt.float32
    half = {"bf16": mybir.dt.bfloat16, "f16": mybir.dt.float16}
    mm_dt = mybir.dt.float32r if dtype == "f32r" else half[dtype]
    # matmul-2 operand dtype: DVE cannot produce float32r, so f32r mode
    # runs the (tiny) second matmul in plain fp32
    v_dt = f32 if dtype == "f32r" else half[dtype]

    WKC = M + NP  # 832 cols per k-tile: [w(256) | kvf(576)]

    nc = bacc.Bacc()
    wk_d = nc.declare_dram_parameter("wk", [128, KT, WKC], mm_dt, isOutput=False)
    cwbc_d = nc.declare_dram_parameter("cwbc", [128, 131], f32, isOutput=False)
    z_d = nc.declare_dram_parameter("zout", [128, SLOTS, NP], f32, isOutput=True)

    with TileContext(nc) as tc:
        with (
            tc.tile_pool(name="consts", bufs=1) as cpool,
            tc.tile_pool(name="wbig", bufs=1) as wpool,
            tc.tile_pool(name="vbuf", bufs=1) as vpool,
            tc.tile_pool(name="zbuf", bufs=1) as zpool,
            tc.tile_pool(name="ps1", bufs=1, space="PSUM") as ps1,
            tc.tile_pool(name="ps2", bufs=3, space="PSUM") as ps2,
            tc.tile_pool(name="ps0", bufs=1, space="PSUM") as ps0,
        ):
            # PE warm-up: dummy matmuls on a zeroed scratch tile keep the
            # PE_HAM activity window busy from t~0 so real matmuls run at
            # 2.4 GHz instead of the cold 1.2 GHz
            warm_t = cpool.tile([128, 512], f32, name="warm_t")
            nc.gpsimd.memset(warm_t[:], 0.0)
            wps = ps0.tile([128, 512], f32, name="wps")
            for _ in range(4):
                nc.tensor.matmul(
                    wps[:], lhsT=warm_t[:, 0:128], rhs=warm_t[:],
                    start=True, stop=True,
                )

            cwbc_t = cpool.tile([128, 131], f32, name="cwbc_t")
            nc.sync.dma_start(cwbc_t[:], cwbc_d[:])
            # DVE-produced copy of conv_w.T so matmul-2 waits only on DVE
            cw_t = cpool.tile([128, 128], v_dt, name="cw_t")
            nc.vector.tensor_copy(cw_t[:], cwbc_t[:, 0:128])

            wk_t = wpool.tile([128, KT, WKC], mm_dt, name="wk_t")
            # chunked loads, small first so the first matmul starts early
            sizes = [1, 2, 3, 3, 4, 4, 4, 4]
            lo = 0
            for sz in sizes:
                nc.sync.dma_start(wk_t[:, lo:lo + sz, :], wk_d[:, lo:lo + sz, :])
                lo += sz

            # matmul 1: V[f, n] = sum_j WvT[j, f] * KVF_T[j, n]
            ps_v = [
                [ps1.tile([128, NCHUNK], f32, name=f"psv{m}{n}") for n in range(2)]
                for m in range(2)
            ]
            for k in range(KT):
                for m in range(2):
                    for n in range(2):
                        nc.tensor.matmul(
                            ps_v[m][n][:],
                            lhsT=wk_t[:, k, m * 128:(m + 1) * 128],
                            rhs=wk_t[:, k, M + n * NCHUNK:M + (n + 1) * NCHUNK],
                            start=(k == 0),
                            stop=(k == KT - 1),
                        )
                # keep-warm filler: PE would otherwise idle waiting for the
                # next k-tile DMA, letting PE_HAM throttle the clock to 1.2GHz.
                # Small moving dim: just enough activity to hold the clock.
                if k % 2 == 0:
                    nc.tensor.matmul(
                        wps[:, 0:128], lhsT=warm_t[:, 0:128],
                        rhs=warm_t[:, 0:128],
                        start=True, stop=True,
                    )

            # V to SBUF (+bv), zero-padded to 128 partitions per pixel-slot
            v_t = []
            for s in range(SLOTS):
                vt = vpool.tile([128, NP], v_dt, name=f"vt{s}")
                nc.vector.memset(vt[64:128, :], 0.0)
                v_t.append(vt)
            for m in range(2):
                for n in range(2):
                    for h in range(2):
                        s = 2 * m + h
                        nc.vector.tensor_tensor(
                            out=v_t[s][0:64, n * NCHUNK:(n + 1) * NCHUNK],
                            in0=ps_v[m][n][h * 64:(h + 1) * 64, :],
                            in1=cwbc_t[h * 64:(h + 1) * 64, 128 + m:129 + m]
                            .to_broadcast((64, NCHUNK)),
                            op=mybir.AluOpType.add,
                        )

            # matmul 2: z[o2, n] = sum_o conv_w[o2, o] * V[s*64+o, n]
            z_t = zpool.tile([128, SLOTS, NP], f32, name="z_t")
            for s in range(SLOTS):
                for n in range(2):
                    nsl = slice(n * NCHUNK, (n + 1) * NCHUNK)
                    psz = ps2.tile([128, NCHUNK], f32, name="psz")
                    nc.tensor.matmul(
                        psz[:],
                        lhsT=cw_t[:],
                        rhs=v_t[s][:, nsl],
                        start=True,
                        stop=True,
                    )
                    nc.vector.tensor_tensor(
                        out=z_t[:, s, nsl],
                        in0=psz[:],
                        in1=cwbc_t[:, 130:131].to_broadcast((128, NCHUNK)),
                        op=mybir.AluOpType.add,
                    )
                    # store each chunk as soon as it is ready
                    nc.sync.dma_start(z_d[:, s, nsl], z_t[:, s, nsl])
    nc.finalize()
    return nc


def _get_program(dtype=DTYPE):
    if dtype not in _PROGRAM:
        _PROGRAM[dtype] = _build_program(dtype)
    return _PROGRAM[dtype]


def _round_fp32r(a):
    """Round fp32 array to the FP32R grid (12-bit mantissa): (u+0x800)&~0xfff."""
    u = np.ascontiguousarray(a, dtype=np.float32).view(np.uint32)
    u = (u + np.uint32(0x800)) & np.uint32(0xFFFFF000)
    return u.view(np.float32)


def _mm_cast(a, dtype):
    if dtype == "f32r":
        return _round_fp32r(a)
    if dtype == "f16":
        return np.ascontiguousarray(a, dtype=np.float32).astype(np.float16)
    import ml_dtypes

    return np.ascontiguousarray(a, dtype=np.float32).astype(ml_dtypes.bfloat16)


def _prep_in_maps(x_kv, Wv, bv, conv_w, conv_b, dtype=DTYPE):
    """Host-side shard/layout prep. Returns list of per-core input dicts."""
    x_kv = np.ascontiguousarray(np.asarray(x_kv, dtype=np.float32))
    Wv = np.asarray(Wv, dtype=np.float32)
    bv = np.asarray(bv, dtype=np.float32)
    conv_w = np.asarray(conv_w, dtype=np.float32)
    conv_b = np.asarray(conv_b, dtype=np.float32)

    # gather all 5x5 patches (padded coords: top-left of patch (pi,pj) is
    # original coords (pi*36-2, pj*36-2))
    pad = np.zeros((B, CKV, HW_ + 2 * E, HW_ + 2 * E), np.float32)
    pad[:, :, E:HW_ + E, E:HW_ + E] = x_kv
    r = (np.arange(PI)[:, None] * STRIDE + np.arange(PP)).ravel()  # (60,)
    g = pad[:, :, r[:, None], r[None, :]]                # (B, C, 60, 60)
    g = g.reshape(B, CKV, PI, PP, PI, PP)
    # feature j = c*25 + pr*5 + pc ; patch n = b*144 + pi*12 + pj
    kvf_t = g.transpose(1, 3, 5, 0, 2, 4).reshape(KF, NP)   # (3200, 576)
    # device layout [partition, k-tile, patch]
    kvf_arr = kvf_t.reshape(KT, 128, NP).transpose(1, 0, 2)

    cw = np.zeros((128, 128), np.float32)
    cw[:OUT, :] = conv_w.T  # cw[o, o2] = conv_w[o2, o]

    in_maps = []
    for c in range(NCORES):
        perm = np.array(
            [o * PP * PP + s for s in S_LISTS[c] for o in range(OUT)], np.int64
        )
        wv_t = Wv[perm].T                      # (3200, 256)
        wv_arr = wv_t.reshape(KT, 128, M).transpose(1, 0, 2)
        # single blob: per k-tile [w(256) | kvf(576)]
        wk = np.concatenate([wv_arr, kvf_arr], axis=2)  # (128, 25, 832)
        wk = _mm_cast(wk, dtype)
        # f32 consts blob: [cw(128) | bv(2) | cb(1)]
        cwbc = np.empty((128, 131), np.float32)
        cwbc[:, 0:128] = cw
        cwbc[:, 128:130] = bv[perm].reshape(2, 128).T
        cwbc[:, 130] = conv_b
        in_maps.append({"wk": wk, "cwbc": cwbc})
    return in_maps


def _assemble(z_list, conv_b, out_dtype=np.float32):
    """Scatter per-core z outputs into the full (B, 128, 432, 432) map."""
    conv_b = np.asarray(conv_b, dtype=np.float32)
    y = np.empty((B, O2, HW_, HW_), np.float32)
    y[:] = conv_b.reshape(1, O2, 1, 1)
    base = np.arange(PI) * STRIDE
    for c in range(NCORES):
        z = z_list[c]  # (128, SLOTS, 576)
        for t in range(VALID[c]):
            s = S_LISTS[c][t]
            pr, pc = divmod(s, PP)
            blk = z[:, t, :].reshape(O2, B, PI, PI).transpose(1, 0, 2, 3)
            y[:, :, (base + pr)[:, None], (base + pc)[None, :]] = blk
    return y.astype(out_dtype, copy=False)


def _run(inputs, trace=False, trace_kwargs=None, dtype=DTYPE):
    from concourse.bass_utils import run_bass_kernel_spmd

    in_maps = _prep_in_maps(
        inputs["x_kv"], inputs["Wv"], inputs["bv"],
        inputs["conv_w"], inputs["conv_b"], dtype=dtype,
    )
    nc = _get_program(dtype)
    kw = {}
    if trace:
        kw["trace"] = True
        if trace_kwargs:
            kw.update(trace_kwargs)
    res = run_bass_kernel_spmd(nc, in_maps, list(range(NCORES)), **kw)
    z_list = [res.results[c]["zout"] for c in range(NCORES)]
    out = _assemble(z_list, inputs["conv_b"])
    return out, res


def kernel(**inputs):
    out, _ = _run(inputs, trace=False)
    return out



# revision 4
# speedup vs baseline: 1.1784x; 1.1784x over previous
# Trainium2 Bass kernel for nn_LocalCrossAttentionModule.
#
# Math: softmax over a size-1 axis is identically 1, so q/k (and x_query,
# Wq, bq, Wk, bk) never affect the output. The module reduces to, per
# 5x5 patch p (576 of them = 4 batch x 12x12 grid, stride 36):
#   kvf_p  = flatten(x_kv patch)                  (3200,)
#   v_p    = Wv @ kvf_p + bv                      (1600,) viewed as (64, 5, 5)
#   z_p    = conv_w @ v_p[:, s] + conv_b          (128,) per pixel s in 5x5
# z_p is scattered into an otherwise-constant (conv_b) output map.
#
# Sharding: 4 feature-shards x 2 patch-halves across 8 cores. Each
# feature-shard owns 6 whole patch pixels (24 of 25); the 25th pixel is
# handled by folding the 1x1 conv into the weights host-side
# (W2 = conv_w @ Wv_p24), splitting its 128 conv-output rows 32 per
# feature-shard. Per-core device work is one fused [128, 25, 704] f16
# stream (weights 416 cols | kvf half 288 cols per k-tile).
#
# The device program is raw bacc (no TileContext): Tile's end-of-kernel
# semaphore butterfly costs ~10us of HW exec time, so semaphores are
# placed by hand (7 sems).

import numpy as np

B = 4
CKV = 128
HW_ = 432
E = 2
PP = 5           # patch side
STRIDE = 36
PI = 12          # patch grid side
NP = B * PI * PI      # 576 patches
KF = CKV * PP * PP    # 3200 kv features per patch
KT = KF // 128        # 25 contraction k-tiles
OUT = 64
O2 = 128
NCORES = 8
NF = 4           # feature shards
NPX = 6          # whole pixels per feature shard
WCOLS = NPX * OUT + 32   # 416 weight cols (384 v-rows + 32 folded z-rows)
NCH = NP // 2    # 288 patches per core (half)
WKC = WCOLS + NCH        # 704
CHUNKS = [1, 2, 3, 4, 4, 4, 3, 2, 1, 1]  # k-tiles per input DMA (sum 25)
N_WARM = 14      # PE warm-up matmuls (cold ~240ns each ~ 3.4us window)

_PROGRAM = {}


def _build_program():
    import concourse.mybir as mybir
    from concourse import bacc

    f32 = mybir.dt.float32
    f16 = mybir.dt.float16
    add = mybir.AluOpType.add

    nc = bacc.Bacc()
    wk_d = nc.declare_dram_parameter("wk", [128, KT, WKC], f16, isOutput=False)
    cb_d = nc.declare_dram_parameter("cb", [128, 5], f32, isOutput=False)
    cw_d = nc.declare_dram_parameter("cw", [128, 128], f16, isOutput=False)
    z_d = nc.declare_dram_parameter("z", [128, NPX, NCH], f16, isOutput=True)
    z24_d = nc.declare_dram_parameter("z24", [32, NCH], f16, isOutput=True)

    # chunk index that must be complete before k-tile k is consumed
    need = []
    for ci, sz in enumerate(CHUNKS):
        need += [ci] * sz

    from contextlib import ExitStack

    with ExitStack() as stack:
        ec = stack.enter_context
        s_in = ec(nc.semaphore("s_in"))      # wk chunk completions (x16)
        s_c = ec(nc.semaphore("s_c"))        # const DMA completions (x16)
        s_warm = ec(nc.semaphore("s_warm"))  # warm tile memset done
        s_pe = ec(nc.semaphore("s_pe"))      # mm1 accumulation done per m
        s_v = ec(nc.semaphore("s_v"))        # V extracted per m
        s_pe2 = ec(nc.semaphore("s_pe2"))    # mm2 done per pixel
        s_z = ec(nc.semaphore("s_z"))        # z written per pixel
        s_z24 = ec(nc.semaphore("s_z24"))    # z24 written
        s_out = ec(nc.semaphore("s_out"))    # output DMA completions (x16)
        wk_t = ec(nc.sbuf_tensor("wk_t", [128, KT, WKC], f16))
        cb_t = ec(nc.sbuf_tensor("cb_t", [128, 5], f32))
        cw_t = ec(nc.sbuf_tensor("cw_t", [128, 128], f16))
        warm_t = ec(nc.sbuf_tensor("warm_t", [128, NCH], f16))
        v_t = ec(nc.sbuf_tensor("v_t", [128, 3, NCH], f16))
        z_t = ec(nc.sbuf_tensor("z_t", [128, NPX, NCH], f16))
        z24_t = ec(nc.sbuf_tensor("z24_t", [32, NCH], f16))
        psv0 = ec(nc.psum_tensor("psv0", [128, NCH], f32))
        psv1 = ec(nc.psum_tensor("psv1", [128, NCH], f32))
        psv2 = ec(nc.psum_tensor("psv2", [128, NCH], f32))
        psv3 = ec(nc.psum_tensor("psv3", [128, NCH], f32))
        psz0 = ec(nc.psum_tensor("psz0", [128, NCH], f32))
        psz1 = ec(nc.psum_tensor("psz1", [128, NCH], f32))
        psz2 = ec(nc.psum_tensor("psz2", [128, NCH], f32))
        psz3 = ec(nc.psum_tensor("psz3", [128, NCH], f32))
        ps_v = [psv0, psv1, psv2, psv3]
        # mm2 output banks: 4 fresh + reuse psv0/psv1 (their V is long
        # extracted by the time pixels 4/5 run, guarded by s_v)
        ps_z = [psz0, psz1, psz2, psz3, psv0, psv1]
        all_sems = [s_in, s_c, s_warm, s_pe, s_v, s_pe2, s_z, s_z24, s_out]

        with nc.Block() as block:

            @block.sync
            def _(sync):
                lo = 0
                for sz in CHUNKS:
                    sync.dma_start(
                        wk_t[:, lo:lo + sz, :], wk_d[:, lo:lo + sz, :]
                    ).then_inc(s_in, 16)
                    lo += sz

            @block.scalar
            def _(scalar):
                scalar.dma_start(cb_t[:], cb_d[:]).then_inc(s_c, 16)
                scalar.dma_start(cw_t[:], cw_d[:]).then_inc(s_c, 16)
                # stores, earliest-ready first
                scalar.wait_ge(s_z24, 1)
                scalar.dma_start(z24_d[:], z24_t[:]).then_inc(s_out, 16)
                scalar.wait_ge(s_z, 2)
                scalar.dma_start(z_d[:, 0:2, :], z_t[:, 0:2, :]).then_inc(s_out, 16)
                scalar.wait_ge(s_z, 4)
                scalar.dma_start(z_d[:, 2:4, :], z_t[:, 2:4, :]).then_inc(s_out, 16)
                scalar.wait_ge(s_z, 6)
                scalar.dma_start(z_d[:, 4:6, :], z_t[:, 4:6, :]).then_inc(s_out, 16)

            @block.tensor
            def _(tensor):
                # warm-up: keeps PE_HAM busy through the first-chunk DMA
                # latency so real matmuls run at 2.4 GHz
                tensor.wait_ge(s_warm, 1)
                for _ in range(N_WARM):
                    tensor.matmul(
                        psz0[:], lhsT=warm_t[:, 0:128], rhs=warm_t[:],
                        start=True, stop=True,
                    )
                # mm1: V[f, n] accumulated over 25 k-tiles
                last_need = -1
                for k in range(KT):
                    if need[k] != last_need:
                        tensor.wait_ge(s_in, 16 * (need[k] + 1))
                        last_need = need[k]
                    for m in range(4):
                        mw = 128 if m < 3 else 32
                        mm = tensor.matmul(
                            ps_v[m][0:mw, :],
                            lhsT=wk_t[:, k, m * 128:m * 128 + mw],
                            rhs=wk_t[:, k, WCOLS:WKC],
                            start=(k == 0),
                            stop=(k == KT - 1),
                        )
                        if k == KT - 1:
                            mm.then_inc(s_pe, 1)
                # mm2: z[o2, n] per pixel, contraction over 64 v-features
                tensor.wait_ge(s_c, 32)
                for p in range(NPX):
                    m, h = divmod(p, 2)
                    if h == 0:
                        tensor.wait_ge(s_v, m + 1)
                    tensor.matmul(
                        ps_z[p][:],
                        lhsT=cw_t[64 * h:64 * (h + 1), :],
                        rhs=v_t[64 * h:64 * (h + 1), m, :],
                        start=True, stop=True,
                    ).then_inc(s_pe2, 1)

            @block.vector
            def _(vector):
                vector.memset(warm_t[:], 0.0).then_inc(s_warm, 1)
                vector.wait_ge(s_c, 16)
                for m in range(3):
                    vector.wait_ge(s_pe, m + 1)
                    vector.tensor_tensor(
                        out=v_t[:, m, :],
                        in0=ps_v[m][:],
                        in1=cb_t[:, m:m + 1].to_broadcast((128, NCH)),
                        op=add,
                    ).then_inc(s_v, 1)
                vector.wait_ge(s_pe, 4)
                vector.tensor_tensor(
                    out=z24_t[:],
                    in0=ps_v[3][0:32, :],
                    in1=cb_t[0:32, 3:4].to_broadcast((32, NCH)),
                    op=add,
                ).then_inc(s_z24, 1)
                for p in range(NPX):
                    vector.wait_ge(s_pe2, p + 1)
                    vector.tensor_tensor(
                        out=z_t[:, p, :],
                        in0=ps_z[p][:],
                        in1=cb_t[:, 4:5].to_broadcast((128, NCH)),
                        op=add,
                    ).then_inc(s_z, 1)

            @block.gpsimd
            def _(gpsimd):
                # end-of-kernel: wait for all stores, then restore sem state
                gpsimd.wait_ge(s_out, 16 * 4)
                nums = sorted(s.num for s in all_sems)
                lo, hi = nums[0], nums[-1]
                assert nums == list(range(lo, hi + 1))
                gpsimd.dma_reset(range(lo, hi + 1))
                gpsimd.sem_clear(range(lo, hi + 1))

    nc.finalize()
    return nc


def _get_program():
    if "p" not in _PROGRAM:
        _PROGRAM["p"] = _build_program()
    return _PROGRAM["p"]


def _prep_in_maps(x_kv, Wv, bv, conv_w, conv_b):
    """Host-side shard/layout prep. Returns list of per-core input dicts."""
    x_kv = np.ascontiguousarray(np.asarray(x_kv, dtype=np.float32))
    Wv = np.asarray(Wv, dtype=np.float32)
    bv = np.asarray(bv, dtype=np.float32)
    conv_w = np.asarray(conv_w, dtype=np.float32)
    conv_b = np.asarray(conv_b, dtype=np.float32)

    # gather all 5x5 patches (padded coords: top-left of patch (pi,pj) is
    # original coords (pi*36-2, pj*36-2))
    pad = np.zeros((B, CKV, HW_ + 2 * E, HW_ + 2 * E), np.float32)
    pad[:, :, E:HW_ + E, E:HW_ + E] = x_kv
    r = (np.arange(PI)[:, None] * STRIDE + np.arange(PP)).ravel()  # (60,)
    g = pad[:, :, r[:, None], r[None, :]]                # (B, C, 60, 60)
    g = g.reshape(B, CKV, PI, PP, PI, PP)
    # feature j = c*25 + pr*5 + pc ; patch n = b*144 + pi*12 + pj
    kvf_t = g.transpose(1, 3, 5, 0, 2, 4).reshape(KF, NP)     # (3200, 576)
    kv_arr = kvf_t.reshape(KT, 128, NP).transpose(1, 0, 2)    # (128, 25, 576)
    kv_arr = np.ascontiguousarray(kv_arr).astype(np.float16)

    # conv folded into the 25th pixel's weights
    perm24 = np.array([o * PP * PP + 24 for o in range(OUT)], np.int64)
    W2 = conv_w @ Wv[perm24]                 # (128, 3200)
    b2 = conv_w @ bv[perm24] + conv_b        # (128,)

    # conv_w.T duplicated into both partition halves (mm2 lhsT must share
    # the rhs base partition)
    cw = np.ascontiguousarray(
        np.concatenate([conv_w.T, conv_w.T], axis=0)).astype(np.float16)

    in_maps = [None] * NCORES
    for f in range(NF):
        pixels = range(NPX * f, NPX * (f + 1))
        perm = np.array(
            [o * PP * PP + s for s in pixels for o in range(OUT)], np.int64
        )  # 384, layout j = s_local*64 + o
        A = np.concatenate([Wv[perm], W2[32 * f:32 * (f + 1)]], axis=0)  # (416, 3200)
        lhsT = np.ascontiguousarray(A.T)                     # (3200, 416)
        w_arr = lhsT.reshape(KT, 128, WCOLS).transpose(1, 0, 2)  # (128, 25, 416)
        w_arr = np.ascontiguousarray(w_arr).astype(np.float16)

        cb = np.zeros((128, 5), np.float32)
        cb[:, 0:3] = bv[perm].reshape(3, 128).T
        cb[0:32, 3] = b2[32 * f:32 * (f + 1)]
        cb[:, 4] = conv_b

        for p in range(2):
            wk = np.concatenate(
                [w_arr, kv_arr[:, :, NCH * p:NCH * (p + 1)]], axis=2
            )  # (128, 25, 704) f16
            in_maps[2 * f + p] = {
                "wk": np.ascontiguousarray(wk),
                "cb": cb,
                "cw": cw,
            }
    return in_maps


def _assemble(results, conv_b, out_dtype=np.float32):
    """Scatter per-core z outputs into the full (B, 128, 432, 432) map."""
    conv_b = np.asarray(conv_b, dtype=np.float32)
    y = np.empty((B, O2, HW_, HW_), np.float32)
    y[:] = conv_b.reshape(1, O2, 1, 1)
    base = np.arange(PI) * STRIDE
    for c in range(NCORES):
        f, p = divmod(c, 2)
        bs = slice(2 * p, 2 * p + 2)  # patch half p covers batches 2p, 2p+1
        z = np.asarray(results[c]["z"], np.float32)      # (128, 6, 288)
        for sl, s in enumerate(range(NPX * f, NPX * (f + 1))):
            pr, pc = divmod(s, PP)
            blk = z[:, sl, :].reshape(O2, 2, PI, PI).transpose(1, 0, 2, 3)
            y[bs, :, (base + pr)[:, None], (base + pc)[None, :]] = blk
        z24 = np.asarray(results[c]["z24"], np.float32)  # (32, 288)
        blk = z24.reshape(32, 2, PI, PI).transpose(1, 0, 2, 3)
        y[bs, 32 * f:32 * (f + 1),
          (base + PP - 1)[:, None], (base + PP - 1)[None, :]] = blk
    return y.astype(out_dtype, copy=False)


def _run(inputs, trace=False, trace_kwargs=None):
    from concourse.bass_utils import run_bass_kernel_spmd

    in_maps = _prep_in_maps(
        inputs["x_kv"], inputs["Wv"], inputs["bv"],
        inputs["conv_w"], inputs["conv_b"],
    )
    nc = _get_program()
    kw = {}
    if trace:
        kw["trace"] = True
        if trace_kwargs:
            kw.update(trace_kwargs)
    res = run_bass_kernel_spmd(nc, in_maps, list(range(NCORES)), **kw)
    out = _assemble(res.results, inputs["conv_b"])
    return out, res


def kernel(**inputs):
    out, _ = _run(inputs, trace=False)
    return out
